# revision 52
# baseline (speedup 1.0000x reference)
"""Trainium2 Bass kernel for nn_Net_25847113187867 (dense_cnn).

The reference slides W = 16384 stride-1 windows over x (1,2,L), runs
conv(s5)/conv(s3)/conv(s2) + 3-layer MLP + hidden-size-1 Elman RNN per
window, twice (second pass with x channel 0 negated), and returns the
antisymmetrized scan outputs (y - y_)/2.

Restructure (v3):
  * Window conv stack == dilated convs over the full sequence; fc3+RNN
    input row folded into one 80->1 vector on the host; conv1 bias
    folded into an ones-row of the input matrix.
  * Pass A and pass B (negated ch0) share one activation tile per conv
    stage: [A; B] stacked across the 128 partitions.  conv2/conv3 use
    block-diagonal [w;0 / 0;w] weights so one matmul per tap computes
    both passes, and c1/c2 evacuate with a single full-width
    [128, cols] relu op.  conv3 evacuates into per-pass [site; site+30]
    stacks so fc1 contracts tap pairs at full 128 depth.
  * Weights packed into 5 dram tensors DMA'd from 3 engine queues in
    parallel, ordered by first use (W2B lands before c1 finishes).
  * Dummy f32r warm-up matmuls on uninitialized scratch ramp the PE
    HAM clock gate (4/8 -> 8/8) before the real work arrives.
  * Matmul column blocks all >= 256 (fp32r runs 4x slower below 256).
  * tanh scan parallelized: 63 chunk rows x 33 outputs with 44-step
    warmup halo + 2 Newton/DEER iterations; pass A and B merged into
    one [128, 77] op chain on the vector engine only (tanh == degree-5
    polynomial, valid for the |z|<=0.3 arguments here).  cur tiles
    carry a zero leading column so the shifted-recurrence scan needs no
    separate B array.  Pass-A scan prep overlaps pass-B fc2 matmuls.
  * 8 cores split outputs into 2048-position slices (overlapping input
    halos, no collectives).  All matmuls in float32r.
"""

import numpy as np

L = 16684
W = 16384
P = 2048            # output positions per core
CH = 33             # scan chunk length (output steps per chunk row)
KW = 44             # per-chunk warmup halo steps (|whh|^44 * 0.33 ~ 3e-6)
SC = KW + CH        # 77 scan columns per chunk row
HALO = KW           # 44: left halo of xp positions per core
NY = 62 * CH + SC + 1  # 2124 xp positions per core: [s-44, s+2080)
NC3 = NY + 180      # 2304 c3 positions per core
NC2 = NC3 + 76      # 2380
NC1 = NC2 + 26      # 2406
NX = NC1 + 6        # 2412
SCAN_ITERS = 2
N_WARMUP = 3        # dummy fp32 matmuls to ramp the PE clock gate


def _groups(n):
    """column groups (<=1024, psum-bank pair) with all sub-blocks in
    [256, 512] so fp32r matmuls run at full rate; everything even."""
    assert n % 2 == 0
    out, o = [], 0
    while o < n:
        rem = n - o
        gw = min(1024, rem)
        if rem > gw and rem - gw < 256:
            gw = rem - 256
        if gw <= 512:
            subs = [(0, gw)]
        elif gw <= 768:
            subs = [(0, gw - 256), (gw - 256, 256)]
        else:
            subs = [(0, 512), (512, gw - 512)]
        out.append((o, gw, subs))
        o += gw
    return out


def _build_program(a_const, c0_const):
    import concourse.bass as bass
    import concourse.mybir as mybir
    import concourse.tile as tile
    from concourse import bacc
    from contextlib import ExitStack

    dt = mybir.dt
    f32 = dt.float32
    AF = mybir.ActivationFunctionType
    OP = mybir.AluOpType
    f32r = dt.float32r
    bf16 = dt.bfloat16

    CB16 = 960 + 320               # PKB cols (bf16-shipped): F1P F1S
    C2 = 160 + 80 + 2              # PK2 cols (f32r): F2 F2S2 VV(pad 2)
    C3 = 2 + 3 + 1 + SC + 1        # PK3 cols: B2AB B3AB FB1 FB2 MASK (+pad)

    nc = bacc.Bacc("TRN2", target_bir_lowering=False, debug=False,
                   num_devices=8)

    xww_d = nc.dram_tensor("xww", [11, NX + 128], f32r, kind="ExternalInput")
    pw2_d = nc.dram_tensor("pw2", [128, 768], bf16, kind="ExternalInput")
    pw3_d = nc.dram_tensor("pw3", [128, 768], bf16, kind="ExternalInput")
    pkb_d = nc.dram_tensor("pkb", [128, CB16], bf16, kind="ExternalInput")
    pk2_d = nc.dram_tensor("pk2", [128, C2], f32r, kind="ExternalInput")
    pk3_d = nc.dram_tensor("pk3", [128, C3], f32, kind="ExternalInput")
    y_d = nc.dram_tensor("y", [1, 63 * CH], f32, kind="ExternalOutput")

    with ExitStack() as ctx:
        tc = ctx.enter_context(tile.TileContext(nc))
        wp = ctx.enter_context(tc.tile_pool(name="weights", bufs=1))
        sp = ctx.enter_context(tc.tile_pool(name="acts", bufs=1))
        pp = ctx.enter_context(tc.tile_pool(name="ps", bufs=4, space="PSUM"))

        WU = wp.tile([128, 192], f32, name="WU", tag="WU")
        XWW = wp.tile([11, NX + 128], f32r, name="xww", tag="xww")
        W2BC = wp.tile([128, 768], bf16, name="pw2", tag="pw2")
        W3BC = wp.tile([128, 768], bf16, name="pw3", tag="pw3")
        PKBC = wp.tile([128, CB16], bf16, name="pkb", tag="pkb")
        W2B = wp.tile([128, 768], f32r, name="w2f", tag="w2f")
        W3B = wp.tile([128, 768], f32r, name="w3f", tag="w3f")
        F1P = wp.tile([128, 960], f32r, name="f1pf", tag="f1pf")
        F1S = wp.tile([64, 320], f32r, name="f1sf", tag="f1sf")
        PK2 = wp.tile([128, C2], f32r, name="pk2", tag="pk2")
        PK3 = wp.tile([128, C3], f32, name="pk3", tag="pk3")

        # warm-up scratch init on gpsimd (earliest engine out of preamble)
        nc.gpsimd.memset(WU[:, :], 0.0)
        # parallel DMA issue across the three DMA-capable engine queues,
        # ordered by first use; conv/fc1 weights ship as bf16 (exact f32r
        # widening below halves the bandwidth-bound weight transfer)
        nc.sync.dma_start(XWW[:], xww_d.ap())
        nc.gpsimd.dma_start(W2BC[:], pw2_d.ap())
        nc.sync.dma_start(PK3[:], pk3_d.ap())
        nc.scalar.dma_start(PKBC[:], pkb_d.ap())
        nc.gpsimd.dma_start(W3BC[:], pw3_d.ap())
        nc.scalar.dma_start(PK2[:], pk2_d.ap())

        # widen bf16 -> f32r (vector copies are exact)
        nc.vector.tensor_copy(W2B[:, :], W2BC[:, :])
        nc.vector.tensor_copy(W3B[:, :], W3BC[:, :])

        XW = XWW[:, 0:NX]
        W1 = XWW[:, NX:NX + 128]
        F2 = PK2[:, 0:160]
        F2S2 = PK2[:, 160:240]
        VV = PK2[:, 240:241]
        B2AB = PK3[:, 0:1]
        B3AB = PK3[:, 1:2]
        FB1 = PK3[:, 2:5]
        FB2 = PK3[:, 5:6]
        MASK = PK3[:, 6:6 + SC]

        SAB = sp.tile([128, NC1], f32r, name="SAB", tag="SAB")
        TAB = sp.tile([128, NC2], f32r, name="TAB", tag="TAB")
        U_ = {"A": sp.tile([128, NC3], f32r, name="UA", tag="UA"),
              "B": sp.tile([128, NC3], f32r, name="UB", tag="UB")}
        Y1 = {("A", 0): sp.tile([128, NY], f32r, name="Y1A0", tag="Y1A0"),
              ("A", 1): sp.tile([128, NY], f32r, name="Y1A1", tag="Y1A1"),
              ("B", 0): sp.tile([128, NY], f32r, name="Y1B0", tag="Y1B0"),
              ("B", 1): sp.tile([128, NY], f32r, name="Y1B1", tag="Y1B1")}
        Y12 = sp.tile([128, NY], f32r, name="Y12", tag="Y12")
        Y2 = {"A": sp.tile([80, NY], f32r, name="Y2A", tag="Y2A"),
              "B": sp.tile([80, NY], f32r, name="Y2B", tag="Y2B")}
        XPR = {"A": sp.tile([1, NY], f32, name="XPRA", tag="XPRA"),
               "B": sp.tile([1, NY], f32, name="XPRB", tag="XPRB")}

        def stile(nm, cols=SC):
            return sp.tile([128, cols], f32, name=nm, tag=nm)

        SCT, ZT, FT, GT, DT, T2T = (
            stile(n) for n in ("SCT", "Z", "F", "G", "DD", "T2"))
        # cur tiles carry a zero leading column: view [:, 1:SC+1] is the
        # value, [:, 0:SC] is the shifted-by-one view
        HT, H2T = stile("H", SC + 1), stile("H2", SC + 1)
        CB = stile("CB", CH)
        D = stile("D", CH)

        # ---------------- warm-up (ramps HAM clock gate) ----------------
        # fp32 matmuls sized to bridge until the input DMA lands, keeping
        # the PE busy so the HAM activity window ramps the clock
        for i in range(N_WARMUP):
            pw = pp.tile([128, 1024], f32, name="ps", tag="ps")
            nc.tensor.matmul(pw[0:32, 0:160], WU[:, 160:192], WU[:, 0:160],
                             start=True, stop=True)

        _ct = [0]

        def evac(out_ap, ps_ap, bias_ap, eng=None):
            """relu(ps + bias) -> out; alternate scalar / vector engines."""
            if eng is None:
                use_act = _ct[0] % 2 == 0
                _ct[0] += 1
            else:
                use_act = eng == "scalar"
            if use_act:
                if bias_ap is None:
                    nc.scalar.activation(out_ap, ps_ap, AF.Relu)
                else:
                    nc.scalar.activation(out_ap, ps_ap, AF.Relu,
                                         bias=bias_ap)
            else:
                if bias_ap is None:
                    nc.vector.tensor_scalar(out_ap, ps_ap, 0.0, None, OP.max)
                else:
                    nc.vector.tensor_scalar(out_ap, ps_ap, bias_ap, 0.0,
                                            OP.add, OP.max)

        # zero the leading columns of the cur tiles (one-time, runs early)
        nc.vector.memset(HT[:, 0:1], 0.0)
        nc.vector.memset(H2T[:, 0:1], 0.0)
        nc.vector.memset(SCT[0:1, :], 0.0)
        nc.vector.memset(SCT[64:65, :], 0.0)

        # ---------------- c1: both passes in one matmul -----------------
        # ps rows 0:64 = c1A, 64:128 = c1B (bias via ones-row of XW)
        for goff, gw, subs in _groups(NC1):
            ps = pp.tile([128, 1024], f32, name="ps", tag="ps")
            for bo, nb in subs:
                o = goff + bo
                nc.tensor.matmul(ps[:, bo:bo + nb], W1[:, :],
                                 XW[:, o:o + nb], start=True, stop=True)
            evac(SAB[:, goff:goff + gw], ps[:, :gw], None)

        # widen the fc1 weights while the conv stages run
        nc.vector.tensor_copy(F1P[:, :], PKBC[:, 0:960])
        nc.vector.tensor_copy(F1S[0:64, :], PKBC[0:64, 960:1280])

        # ------------- c2/c3: block-diagonal dilated convs --------------
        # tap-outer order: the two sub-block accumulations interleave, so
        # only one acc-start bubble per group and each LDW serves 2 mms
        def conv_stage(SRC, n_out, Wt, dil):
            for goff, gw, subs in _groups(n_out):
                ps = pp.tile([128, 1024], f32, name="ps", tag="ps")
                for t in range(6):
                    for bo, nb in subs:
                        o = goff + bo
                        nc.tensor.matmul(
                            ps[:, bo:bo + nb],
                            Wt[:, 128 * t:128 * t + 128],
                            SRC[:, o + dil * t:o + dil * t + nb],
                            start=(t == 0), stop=(t == 5))
                yield goff, gw, ps

        for goff, gw, ps in conv_stage(SAB, NC2, W2B, 5):
            evac(TAB[:, goff:goff + gw], ps[:, :gw], B2AB)

        # c3 evacuates into per-pass [site; site+30] stacks for fc1
        for goff, gw, ps in conv_stage(TAB, NC3, W3B, 15):
            for pX, rows in (("A", slice(0, 64)), ("B", slice(64, 128))):
                dst = U_[pX]
                evac(dst[0:64, goff:goff + gw], ps[rows, :gw],
                     B3AB[rows, :])
                if goff == 0:
                    evac(dst[64:128, 0:gw - 30], ps[rows, 30:gw],
                         B3AB[rows, :])
                else:
                    evac(dst[64:128, goff - 30:goff + gw - 30],
                         ps[rows, :gw], B3AB[rows, :])

        # ---------------- fc1: 448 -> 320 (tap pairs) -------------------
        # weight-outer order: A and B accumulations interleave per chunk,
        # each weight's LDW serves 4 back-to-back matmuls
        for goff, gw, subs in _groups(NY):
            for c in range(2):
                psc = {pX: pp.tile([128, 1024], f32, name="ps", tag="ps")
                       for pX in "AB"}
                for p in range(3):
                    wap = F1P[:, 320 * p + 128 * c:320 * p + 128 * c + 128]
                    for pX in "AB":
                        U = U_[pX]
                        for bo, nb in subs:
                            o = goff + bo
                            nc.tensor.matmul(
                                psc[pX][:, bo:bo + nb], wap,
                                U[:, o + 60 * p:o + 60 * p + nb],
                                start=(p == 0), stop=False)
                for pX in "AB":
                    U = U_[pX]
                    for bo, nb in subs:
                        o = goff + bo
                        nc.tensor.matmul(
                            psc[pX][:, bo:bo + nb],
                            F1S[0:64, 128 * c:128 * c + 128],
                            U[0:64, o + 180:o + 180 + nb],
                            start=False, stop=True)
                for pX in "AB":
                    evac(Y1[(pX, c)][:, goff:goff + gw], psc[pX][:, :gw],
                         FB1[:, c:c + 1])
            # chunk 2 (64 outs): A -> Y12[0:64], B -> Y12[64:128]
            psc = {pX: pp.tile([128, 1024], f32, name="ps", tag="ps")
                   for pX in "AB"}
            for p in range(3):
                wap = F1P[:, 320 * p + 256:320 * p + 320]
                for pX in "AB":
                    U = U_[pX]
                    for bo, nb in subs:
                        o = goff + bo
                        nc.tensor.matmul(
                            psc[pX][0:64, bo:bo + nb], wap,
                            U[:, o + 60 * p:o + 60 * p + nb],
                            start=(p == 0), stop=False)
            for pX in "AB":
                U = U_[pX]
                for bo, nb in subs:
                    o = goff + bo
                    nc.tensor.matmul(
                        psc[pX][0:64, bo:bo + nb], F1S[0:64, 256:320],
                        U[0:64, o + 180:o + 180 + nb],
                        start=False, stop=True)
            for pX, pr in (("A", 0), ("B", 64)):
                evac(Y12[pr:pr + 64, goff:goff + gw], psc[pX][0:64, :gw],
                     FB1[pr:pr + 64, 2:3])

        # ------------- fc2 + xp: all of pass A, then pass B -------------
        gl = _groups(NY)

        def fc2_mm(pX, gi):
            goff, gw, subs = gl[gi]
            ps = pp.tile([128, 1024], f32, name="ps", tag="ps")
            pr = 0 if pX == "A" else 64
            for bo, nb in subs:
                o = goff + bo
                nc.tensor.matmul(ps[:80, bo:bo + nb], F2[:, 0:80],
                                 Y1[(pX, 0)][:, o:o + nb],
                                 start=True, stop=False)
                nc.tensor.matmul(ps[:80, bo:bo + nb], F2[:, 80:160],
                                 Y1[(pX, 1)][:, o:o + nb],
                                 start=False, stop=False)
                nc.tensor.matmul(ps[:80, bo:bo + nb],
                                 F2S2[pr:pr + 64, :],
                                 Y12[pr:pr + 64, o:o + nb],
                                 start=False, stop=True)
            # pass-B evacs pinned to scalar so vector is free for the
            # pass-A scan prep that overlaps this phase
            evac(Y2[pX][:, goff:goff + gw], ps[:80, :gw], FB2[0:80, :],
                 eng="scalar" if pX == "B" else None)

        def xp_mm(pX, gi):
            goff, gw, subs = gl[gi]
            ps2 = pp.tile([128, 1024], f32, name="ps", tag="ps")
            for bo, nb in subs:
                o = goff + bo
                nc.tensor.matmul(ps2[:1, bo:bo + nb], VV[0:80, :],
                                 Y2[pX][:, o:o + nb], start=True, stop=True)
            nc.vector.tensor_scalar(XPR[pX][0:1, goff:goff + gw],
                                    ps2[:1, :gw], float(c0_const), None,
                                    OP.add)

        # ------------- merged A/B chunked tanh scan pieces --------------
        # rows 1:64 = pass A chunks, rows 65:128 = pass B chunks
        def ptanh(out, z, rows):
            """tanh(z) for |z|<=0.35 as z*(1 - t/3 + 2t^2/15), t=z^2."""
            t2 = T2T[rows, :]
            g = GT[rows, :]
            nc.vector.tensor_tensor(t2, z, z, OP.mult)
            nc.vector.tensor_scalar(g, t2, 2.0 / 15.0, -1.0 / 3.0,
                                    OP.mult, OP.add)
            nc.vector.tensor_tensor(g, g, t2, OP.mult)
            nc.vector.scalar_tensor_tensor(out, g, 1.0, z, OP.add, OP.mult)

        def gather(pX, r0, r1, eng):
            # gather xp chunk rows [r0, r1) of this pass into SCT
            rlo = 1 if pX == "A" else 65
            xpr = XPR[pX]
            src = bass.AP(tensor=xpr.tensor,
                          offset=xpr.offset + CH * r0,
                          ap=[[NY, 1], [CH, r1 - r0], [1, SC]])
            eng.dma_start(SCT[rlo + r0:rlo + r1, :], src)

        def scan_prep(lo, hi):
            # mask + initial H = tanh(sct) for a 32-aligned partition range
            half = slice(lo, hi)
            nc.vector.tensor_tensor(SCT[half, :], SCT[half, :],
                                    MASK[half, :], OP.mult)
            ptanh(HT[half, 1:SC + 1], SCT[half, :], half)

        a = float(a_const)

        def scan_iter(cur, nxt):
            # Z = a*cur_shifted + sct  (leading zero col makes col0 = sct0)
            nc.vector.scalar_tensor_tensor(ZT[:, :], cur[:, 0:SC], a,
                                           SCT[:, :], OP.mult, OP.add)
            ptanh(FT[:, :], ZT[:, :], slice(0, 128))
            # G = a*(1 - F^2)
            nc.vector.tensor_tensor(GT[:, :], FT[:, :], FT[:, :], OP.mult)
            nc.vector.tensor_scalar(GT[:, :], GT[:, :], -a, a, OP.mult,
                                    OP.add)
            # d1 = F - G*cur_shifted;  nxt_t = G_t*nxt_{t-1} + d1_t
            nc.vector.tensor_tensor(T2T[:, :], GT[:, :], cur[:, 0:SC],
                                    OP.mult)
            nc.vector.tensor_tensor(DT[:, :], FT[:, :], T2T[:, :],
                                    OP.subtract)
            nc.vector.tensor_tensor_scan(nxt[:, 1:SC + 1], GT[:, :],
                                         DT[:, :], 0.0, OP.mult, OP.add)

        # ---------------- emit fc2/xp + scan schedule -------------------
        # chunk row i (0-based) reads xp cols [33i, 33i+77): i<31 needs
        # xp groups 0-1, the rest all three.  Each masked range consumes
        # at most 2 DMA-written regions (2-semaphore wait limit).
        fc2_mm("A", 0); fc2_mm("A", 1)
        xp_mm("A", 0); xp_mm("A", 1)
        gather("A", 0, 31, nc.gpsimd)
        fc2_mm("A", 2)
        xp_mm("A", 2)
        gather("A", 31, 63, nc.gpsimd)
        scan_prep(0, 64)                    # overlaps pass-B fc2 below
        fc2_mm("B", 0); fc2_mm("B", 1)
        xp_mm("B", 0); xp_mm("B", 1)
        gather("B", 0, 31, nc.sync)
        gather("B", 31, 47, nc.gpsimd)   # rows 31:47 only need groups 0-1
        fc2_mm("B", 2)
        scan_prep(64, 96)
        xp_mm("B", 2)
        gather("B", 47, 63, nc.sync)
        scan_prep(96, 128)

        cur, nxt = HT, H2T
        for it in range(SCAN_ITERS):
            scan_iter(cur, nxt)
            cur, nxt = nxt, cur

        # y = (hA - hB)/2; row r covers outputs 33*(r-1) .. +32
        hfA = cur[0:64, 1 + KW:1 + SC]
        hfB = cur[64:128, 1 + KW:1 + SC]
        nc.vector.tensor_scalar(CB[0:64, :], hfB, 0.5, None, OP.mult)
        nc.vector.scalar_tensor_tensor(D[0:64, :], hfA, 0.5, CB[0:64, :],
                                       OP.mult, OP.subtract)
        nc.sync.dma_start(
            y_d.ap()[0, 0:63 * CH].rearrange("(r c) -> r c", c=CH),
            D[1:64, :])

    nc.compile()
    return nc


def _prep_inputs(inputs):
    """Host-side packing: per-core input dicts."""
    x0 = np.asarray(inputs["x0"], np.float32)[0]
    w1 = np.asarray(inputs["conv1_w"], np.float32)
    b1 = np.asarray(inputs["conv1_b"], np.float32)
    w2 = np.asarray(inputs["conv2_w"], np.float32)
    b2 = np.asarray(inputs["conv2_b"], np.float32)
    w3 = np.asarray(inputs["conv3_w"], np.float32)
    b3 = np.asarray(inputs["conv3_b"], np.float32)
    f1w = np.asarray(inputs["fc1_w"], np.float32)
    f1b = np.asarray(inputs["fc1_b"], np.float32)
    f2w = np.asarray(inputs["fc2_w"], np.float32)
    f2b = np.asarray(inputs["fc2_b"], np.float32)
    f3w = np.asarray(inputs["fc3_w"], np.float32)
    f3b = np.asarray(inputs["fc3_b"], np.float32)
    wih = np.asarray(inputs["rnn_wih"], np.float32)
    whh = np.asarray(inputs["rnn_whh"], np.float32)
    bih = np.asarray(inputs["rnn_bih"], np.float32)
    bhh = np.asarray(inputs["rnn_bhh"], np.float32)

    a = float(whh[0, 0])
    v = (wih @ f3w)[0]
    c0 = float((wih @ f3b + bih + bhh).item())

    # W1 [11, 128]: rows 0..9 conv taps, row 10 = bias (ones-row input)
    W1 = np.zeros((11, 128), np.float32)
    for c in range(2):
        for k in range(5):
            W1[c * 5 + k, 0:64] = w1[:, c, k]
            W1[c * 5 + k, 64:128] = w1[:, c, k] * (-1.0 if c == 0 else 1.0)
    W1[10, 0:64] = b1
    W1[10, 64:128] = b1

    def pack_blockdiag(w):  # (64,64,6) -> [128, 768]
        out = np.zeros((128, 768), np.float32)
        for t in range(6):
            out[0:64, 128 * t:128 * t + 64] = w[:, :, t].T
            out[64:128, 128 * t + 64:128 * t + 128] = w[:, :, t].T
        return out

    W2B = pack_blockdiag(w2)
    W3B = pack_blockdiag(w3)

    f1r = f1w.reshape(320, 64, 7)  # flat index = ch*7 + m
    F1P = np.zeros((128, 960), np.float32)
    for p in range(3):
        F1P[0:64, 320 * p:320 * p + 320] = f1r[:, :, 2 * p].T
        F1P[64:128, 320 * p:320 * p + 320] = f1r[:, :, 2 * p + 1].T
    F1S = np.zeros((128, 320), np.float32)
    F1S[0:64, :] = f1r[:, :, 6].T
    FB1 = np.zeros((128, 3), np.float32)
    FB1[:, 0] = f1b[0:128]
    FB1[:, 1] = f1b[128:256]
    FB1[0:64, 2] = f1b[256:320]
    FB1[64:128, 2] = f1b[256:320]

    F2 = np.zeros((128, 160), np.float32)
    F2[:, 0:80] = f2w[:, 0:128].T
    F2[:, 80:160] = f2w[:, 128:256].T
    F2S2 = np.zeros((128, 80), np.float32)
    F2S2[0:64, :] = f2w[:, 256:320].T
    F2S2[64:128, :] = f2w[:, 256:320].T
    FB2 = np.zeros((128, 1), np.float32)
    FB2[0:80, 0] = f2b
    VVc = np.zeros((128, 2), np.float32)
    VVc[0:80, 0] = v

    import ml_dtypes
    bf = ml_dtypes.bfloat16
    C2 = 160 + 80 + 2
    C3 = 2 + 3 + 1 + SC + 1

    pkb = np.concatenate([F1P, F1S], axis=1).astype(bf)
    pk2 = np.concatenate([F2, F2S2, VVc], axis=1)
    assert pk2.shape == (128, C2)
    W2B = W2B.astype(bf)
    W3B = W3B.astype(bf)

    B2AB = np.concatenate([b2, b2]).reshape(128, 1)
    B3AB = np.concatenate([b3, b3]).reshape(128, 1)

    lpad = HALO
    rpad = (7 * P - HALO + NX + 8) - L
    xpad = np.zeros((2, lpad + L + max(rpad, 0)), np.float32)
    xpad[:, lpad:lpad + L] = x0

    in_maps = []
    for core in range(8):
        s = P * core
        base = lpad + s - HALO
        xww = np.zeros((11, NX + 128), np.float32)
        for c in range(2):
            for k in range(5):
                xww[c * 5 + k, 0:NX] = xpad[c, base + k:base + k + NX]
        xww[10, 0:NX] = 1.0
        xww[:, NX:NX + 128] = W1
        # scan mask: rows 0 and 64 kill garbage; rows r/64+r col j is
        # position s - 44 + 33*(r-1) + j; zero where position < 0
        mask = np.ones((128, SC), np.float32)
        mask[0, :] = 0.0
        mask[64, :] = 0.0
        if core == 0:
            for rr in range(1, 64):
                for j in range(SC):
                    if s - HALO + CH * (rr - 1) + j < 0:
                        mask[rr, j] = 0.0
                        mask[64 + rr, j] = 0.0
        pk3 = np.zeros((128, C3), np.float32)
        pk3[:, 0:1] = B2AB
        pk3[:, 1:2] = B3AB
        pk3[:, 2:5] = FB1
        pk3[:, 5:6] = FB2
        pk3[:, 6:6 + SC] = mask
        in_maps.append(dict(xww=xww, pw2=W2B, pw3=W3B, pkb=pkb, pk2=pk2,
                            pk3=pk3))
    return in_maps, a, c0


LAST_RESULT = None


def kernel(**inputs) -> np.ndarray:
    global LAST_RESULT
    from concourse import bass_utils

    in_maps, a, c0 = _prep_inputs(inputs)
    nc = _build_program(a, c0)
    res = bass_utils.run_bass_kernel_spmd(nc, in_maps, core_ids=list(range(8)))
    LAST_RESULT = res
    out = np.empty((1, W), np.float32)
    for core in range(8):
        out[0, P * core:P * core + P] = res.results[core]["y"][0][:P]
    return out


# revision 53
# speedup vs baseline: 1.0029x; 1.0029x over previous
"""Trainium2 Bass kernel for nn_Net_25847113187867 (dense_cnn).

The reference slides W = 16384 stride-1 windows over x (1,2,L), runs
conv(s5)/conv(s3)/conv(s2) + 3-layer MLP + hidden-size-1 Elman RNN per
window, twice (second pass with x channel 0 negated), and returns the
antisymmetrized scan outputs (y - y_)/2.

Restructure (v3):
  * Window conv stack == dilated convs over the full sequence; fc3+RNN
    input row folded into one 80->1 vector on the host; conv1 bias
    folded into an ones-row of the input matrix.
  * Pass A and pass B (negated ch0) share one activation tile per conv
    stage: [A; B] stacked across the 128 partitions.  conv2/conv3 use
    block-diagonal [w;0 / 0;w] weights so one matmul per tap computes
    both passes, and c1/c2 evacuate with a single full-width
    [128, cols] relu op.  conv3 evacuates into per-pass [site; site+30]
    stacks so fc1 contracts tap pairs at full 128 depth.
  * Weights packed into 5 dram tensors DMA'd from 3 engine queues in
    parallel, ordered by first use (W2B lands before c1 finishes).
  * Dummy f32r warm-up matmuls on uninitialized scratch ramp the PE
    HAM clock gate (4/8 -> 8/8) before the real work arrives.
  * Matmul column blocks all >= 256 (fp32r runs 4x slower below 256).
  * tanh scan parallelized: 63 chunk rows x 33 outputs with 44-step
    warmup halo + 2 Newton/DEER iterations; pass A and B merged into
    one [128, 77] op chain on the vector engine only (tanh == degree-5
    polynomial, valid for the |z|<=0.3 arguments here).  cur tiles
    carry a zero leading column so the shifted-recurrence scan needs no
    separate B array.  Pass-A scan prep overlaps pass-B fc2 matmuls.
  * 8 cores split outputs into 2048-position slices (overlapping input
    halos, no collectives).  All matmuls in float32r.
"""

import numpy as np

L = 16684
W = 16384
P = 2048            # output positions per core
CH = 33             # scan chunk length (output steps per chunk row)
KW = 44             # per-chunk warmup halo steps (|whh|^44 * 0.33 ~ 3e-6)
SC = KW + CH        # 77 scan columns per chunk row
HALO = KW           # 44: left halo of xp positions per core
NY = 62 * CH + SC + 1  # 2124 xp positions per core: [s-44, s+2080)
NC3 = NY + 180      # 2304 c3 positions per core
NC2 = NC3 + 76      # 2380
NC1 = NC2 + 26      # 2406
NX = NC1 + 6        # 2412
SCAN_ITERS = 2
N_WARMUP = 9        # dummy fp32 matmuls to ramp the PE clock gate


def _groups(n):
    """column groups (<=1024, psum-bank pair) with all sub-blocks in
    [256, 512] so fp32r matmuls run at full rate; everything even."""
    assert n % 2 == 0
    out, o = [], 0
    while o < n:
        rem = n - o
        gw = min(1024, rem)
        if rem > gw and rem - gw < 256:
            gw = rem - 256
        if gw <= 512:
            subs = [(0, gw)]
        elif gw <= 768:
            subs = [(0, gw - 256), (gw - 256, 256)]
        else:
            subs = [(0, 512), (512, gw - 512)]
        out.append((o, gw, subs))
        o += gw
    return out


def _build_program(a_const, c0_const):
    import concourse.bass as bass
    import concourse.mybir as mybir
    import concourse.tile as tile
    from concourse import bacc
    from contextlib import ExitStack

    dt = mybir.dt
    f32 = dt.float32
    AF = mybir.ActivationFunctionType
    OP = mybir.AluOpType
    f32r = dt.float32r
    bf16 = dt.bfloat16

    CB16 = 960 + 320               # PKB cols (bf16-shipped): F1P F1S
    C2 = 160 + 80 + 2              # PK2 cols (f32r): F2 F2S2 VV(pad 2)
    C3 = 2 + 3 + 1 + SC + 1        # PK3 cols: B2AB B3AB FB1 FB2 MASK (+pad)

    nc = bacc.Bacc("TRN2", target_bir_lowering=False, debug=False,
                   num_devices=8)

    xww_d = nc.dram_tensor("xww", [11, NX + 128], f32r, kind="ExternalInput")
    pw2_d = nc.dram_tensor("pw2", [128, 768], bf16, kind="ExternalInput")
    pw3_d = nc.dram_tensor("pw3", [128, 768], bf16, kind="ExternalInput")
    pkb_d = nc.dram_tensor("pkb", [128, CB16], bf16, kind="ExternalInput")
    pk2_d = nc.dram_tensor("pk2", [128, C2], f32r, kind="ExternalInput")
    pk3_d = nc.dram_tensor("pk3", [128, C3], f32, kind="ExternalInput")
    y_d = nc.dram_tensor("y", [1, 63 * CH], f32, kind="ExternalOutput")

    with ExitStack() as ctx:
        tc = ctx.enter_context(tile.TileContext(nc))
        wp = ctx.enter_context(tc.tile_pool(name="weights", bufs=1))
        sp = ctx.enter_context(tc.tile_pool(name="acts", bufs=1))
        pp = ctx.enter_context(tc.tile_pool(name="ps", bufs=4, space="PSUM"))

        WU = wp.tile([128, 192], f32, name="WU", tag="WU")
        XWW = wp.tile([11, NX + 128], f32r, name="xww", tag="xww")
        W2BC = wp.tile([128, 768], bf16, name="pw2", tag="pw2")
        W3BC = wp.tile([128, 768], bf16, name="pw3", tag="pw3")
        PKBC = wp.tile([128, CB16], bf16, name="pkb", tag="pkb")
        W2B = wp.tile([128, 768], f32r, name="w2f", tag="w2f")
        W3B = wp.tile([128, 768], f32r, name="w3f", tag="w3f")
        F1P = wp.tile([128, 960], f32r, name="f1pf", tag="f1pf")
        F1S = wp.tile([64, 320], f32r, name="f1sf", tag="f1sf")
        PK2 = wp.tile([128, C2], f32r, name="pk2", tag="pk2")
        PK3 = wp.tile([128, C3], f32, name="pk3", tag="pk3")

        # warm-up scratch init on gpsimd (earliest engine out of preamble)
        nc.gpsimd.memset(WU[:, :], 0.0)
        # parallel DMA issue across the three DMA-capable engine queues,
        # ordered by first use; conv/fc1 weights ship as bf16 (exact f32r
        # widening below halves the bandwidth-bound weight transfer)
        nc.sync.dma_start(XWW[:], xww_d.ap())
        nc.gpsimd.dma_start(W2BC[:], pw2_d.ap())
        nc.sync.dma_start(PK3[:], pk3_d.ap())
        nc.scalar.dma_start(PKBC[:], pkb_d.ap())
        nc.gpsimd.dma_start(W3BC[:], pw3_d.ap())
        nc.scalar.dma_start(PK2[:], pk2_d.ap())

        # widen bf16 -> f32r (vector copies are exact)
        nc.vector.tensor_copy(W2B[:, :], W2BC[:, :])
        nc.vector.tensor_copy(W3B[:, :], W3BC[:, :])

        XW = XWW[:, 0:NX]
        W1 = XWW[:, NX:NX + 128]
        F2 = PK2[:, 0:160]
        F2S2 = PK2[:, 160:240]
        VV = PK2[:, 240:241]
        B2AB = PK3[:, 0:1]
        B3AB = PK3[:, 1:2]
        FB1 = PK3[:, 2:5]
        FB2 = PK3[:, 5:6]
        MASK = PK3[:, 6:6 + SC]

        SAB = sp.tile([128, NC1], f32r, name="SAB", tag="SAB")
        TAB = sp.tile([128, NC2], f32r, name="TAB", tag="TAB")
        U_ = {"A": sp.tile([128, NC3], f32r, name="UA", tag="UA"),
              "B": sp.tile([128, NC3], f32r, name="UB", tag="UB")}
        Y1 = {("A", 0): sp.tile([128, NY], f32r, name="Y1A0", tag="Y1A0"),
              ("A", 1): sp.tile([128, NY], f32r, name="Y1A1", tag="Y1A1"),
              ("B", 0): sp.tile([128, NY], f32r, name="Y1B0", tag="Y1B0"),
              ("B", 1): sp.tile([128, NY], f32r, name="Y1B1", tag="Y1B1")}
        Y12 = sp.tile([128, NY], f32r, name="Y12", tag="Y12")
        Y2 = {"A": sp.tile([80, NY], f32r, name="Y2A", tag="Y2A"),
              "B": sp.tile([80, NY], f32r, name="Y2B", tag="Y2B")}
        XPR = {"A": sp.tile([1, NY], f32, name="XPRA", tag="XPRA"),
               "B": sp.tile([1, NY], f32, name="XPRB", tag="XPRB")}

        def stile(nm, cols=SC):
            return sp.tile([128, cols], f32, name=nm, tag=nm)

        SCT, ZT, FT, GT, DT, T2T = (
            stile(n) for n in ("SCT", "Z", "F", "G", "DD", "T2"))
        # cur tiles carry a zero leading column: view [:, 1:SC+1] is the
        # value, [:, 0:SC] is the shifted-by-one view
        HT, H2T = stile("H", SC + 1), stile("H2", SC + 1)
        CB = stile("CB", CH)
        D = stile("D", CH)

        # ---------------- warm-up (ramps HAM clock gate) ----------------
        # fp32 matmuls sized to bridge until the input DMA lands, keeping
        # the PE busy so the HAM activity window ramps the clock
        for i in range(N_WARMUP):
            pw = pp.tile([128, 1024], f32, name="ps", tag="ps")
            nc.tensor.matmul(pw[0:32, 0:160], WU[:, 160:192], WU[:, 0:160],
                             start=True, stop=True)

        _ct = [0]

        def evac(out_ap, ps_ap, bias_ap, eng=None):
            """relu(ps + bias) -> out; alternate scalar / vector engines."""
            if eng is None:
                use_act = _ct[0] % 2 == 0
                _ct[0] += 1
            else:
                use_act = eng == "scalar"
            if use_act:
                if bias_ap is None:
                    nc.scalar.activation(out_ap, ps_ap, AF.Relu)
                else:
                    nc.scalar.activation(out_ap, ps_ap, AF.Relu,
                                         bias=bias_ap)
            else:
                if bias_ap is None:
                    nc.vector.tensor_scalar(out_ap, ps_ap, 0.0, None, OP.max)
                else:
                    nc.vector.tensor_scalar(out_ap, ps_ap, bias_ap, 0.0,
                                            OP.add, OP.max)

        # zero the leading columns of the cur tiles (one-time, runs early)
        nc.vector.memset(HT[:, 0:1], 0.0)
        nc.vector.memset(H2T[:, 0:1], 0.0)
        nc.vector.memset(SCT[0:1, :], 0.0)
        nc.vector.memset(SCT[64:65, :], 0.0)

        # ---------------- c1: both passes in one matmul -----------------
        # ps rows 0:64 = c1A, 64:128 = c1B (bias via ones-row of XW)
        for goff, gw, subs in _groups(NC1):
            ps = pp.tile([128, 1024], f32, name="ps", tag="ps")
            for bo, nb in subs:
                o = goff + bo
                nc.tensor.matmul(ps[:, bo:bo + nb], W1[:, :],
                                 XW[:, o:o + nb], start=True, stop=True)
            evac(SAB[:, goff:goff + gw], ps[:, :gw], None)

        # widen the fc1 weights while the conv stages run
        nc.vector.tensor_copy(F1P[:, :], PKBC[:, 0:960])
        nc.vector.tensor_copy(F1S[0:64, :], PKBC[0:64, 960:1280])

        # ------------- c2/c3: block-diagonal dilated convs --------------
        # tap-outer order: the two sub-block accumulations interleave, so
        # only one acc-start bubble per group and each LDW serves 2 mms
        def conv_stage(SRC, n_out, Wt, dil):
            for goff, gw, subs in _groups(n_out):
                ps = pp.tile([128, 1024], f32, name="ps", tag="ps")
                for t in range(6):
                    for bo, nb in subs:
                        o = goff + bo
                        nc.tensor.matmul(
                            ps[:, bo:bo + nb],
                            Wt[:, 128 * t:128 * t + 128],
                            SRC[:, o + dil * t:o + dil * t + nb],
                            start=(t == 0), stop=(t == 5))
                yield goff, gw, ps

        for goff, gw, ps in conv_stage(SAB, NC2, W2B, 5):
            evac(TAB[:, goff:goff + gw], ps[:, :gw], B2AB)

        # c3 evacuates into per-pass [site; site+30] stacks for fc1
        for goff, gw, ps in conv_stage(TAB, NC3, W3B, 15):
            for pX, rows in (("A", slice(0, 64)), ("B", slice(64, 128))):
                dst = U_[pX]
                evac(dst[0:64, goff:goff + gw], ps[rows, :gw],
                     B3AB[rows, :])
                if goff == 0:
                    evac(dst[64:128, 0:gw - 30], ps[rows, 30:gw],
                         B3AB[rows, :])
                else:
                    evac(dst[64:128, goff - 30:goff + gw - 30],
                         ps[rows, :gw], B3AB[rows, :])

        # ---------------- fc1: 448 -> 320 (tap pairs) -------------------
        # weight-outer order: A and B accumulations interleave per chunk,
        # each weight's LDW serves 4 back-to-back matmuls
        for goff, gw, subs in _groups(NY):
            for c in range(2):
                psc = {pX: pp.tile([128, 1024], f32, name="ps", tag="ps")
                       for pX in "AB"}
                for p in range(3):
                    wap = F1P[:, 320 * p + 128 * c:320 * p + 128 * c + 128]
                    for pX in "AB":
                        U = U_[pX]
                        for bo, nb in subs:
                            o = goff + bo
                            nc.tensor.matmul(
                                psc[pX][:, bo:bo + nb], wap,
                                U[:, o + 60 * p:o + 60 * p + nb],
                                start=(p == 0), stop=False)
                for pX in "AB":
                    U = U_[pX]
                    for bo, nb in subs:
                        o = goff + bo
                        nc.tensor.matmul(
                            psc[pX][:, bo:bo + nb],
                            F1S[0:64, 128 * c:128 * c + 128],
                            U[0:64, o + 180:o + 180 + nb],
                            start=False, stop=True)
                for pX in "AB":
                    evac(Y1[(pX, c)][:, goff:goff + gw], psc[pX][:, :gw],
                         FB1[:, c:c + 1])
            # chunk 2 (64 outs): A -> Y12[0:64], B -> Y12[64:128]
            psc = {pX: pp.tile([128, 1024], f32, name="ps", tag="ps")
                   for pX in "AB"}
            for p in range(3):
                wap = F1P[:, 320 * p + 256:320 * p + 320]
                for pX in "AB":
                    U = U_[pX]
                    for bo, nb in subs:
                        o = goff + bo
                        nc.tensor.matmul(
                            psc[pX][0:64, bo:bo + nb], wap,
                            U[:, o + 60 * p:o + 60 * p + nb],
                            start=(p == 0), stop=False)
            for pX in "AB":
                U = U_[pX]
                for bo, nb in subs:
                    o = goff + bo
                    nc.tensor.matmul(
                        psc[pX][0:64, bo:bo + nb], F1S[0:64, 256:320],
                        U[0:64, o + 180:o + 180 + nb],
                        start=False, stop=True)
            for pX, pr in (("A", 0), ("B", 64)):
                evac(Y12[pr:pr + 64, goff:goff + gw], psc[pX][0:64, :gw],
                     FB1[pr:pr + 64, 2:3])

        # ------------- fc2 + xp: all of pass A, then pass B -------------
        gl = _groups(NY)

        def fc2_mm(pX, gi):
            goff, gw, subs = gl[gi]
            ps = pp.tile([128, 1024], f32, name="ps", tag="ps")
            pr = 0 if pX == "A" else 64
            for bo, nb in subs:
                o = goff + bo
                nc.tensor.matmul(ps[:80, bo:bo + nb], F2[:, 0:80],
                                 Y1[(pX, 0)][:, o:o + nb],
                                 start=True, stop=False)
                nc.tensor.matmul(ps[:80, bo:bo + nb], F2[:, 80:160],
                                 Y1[(pX, 1)][:, o:o + nb],
                                 start=False, stop=False)
                nc.tensor.matmul(ps[:80, bo:bo + nb],
                                 F2S2[pr:pr + 64, :],
                                 Y12[pr:pr + 64, o:o + nb],
                                 start=False, stop=True)
            # pass-B evacs pinned to scalar so vector is free for the
            # pass-A scan prep that overlaps this phase
            evac(Y2[pX][:, goff:goff + gw], ps[:80, :gw], FB2[0:80, :],
                 eng="scalar" if pX == "B" else None)

        def xp_mm(pX, gi):
            goff, gw, subs = gl[gi]
            ps2 = pp.tile([128, 1024], f32, name="ps", tag="ps")
            for bo, nb in subs:
                o = goff + bo
                nc.tensor.matmul(ps2[:1, bo:bo + nb], VV[0:80, :],
                                 Y2[pX][:, o:o + nb], start=True, stop=True)
            nc.vector.tensor_scalar(XPR[pX][0:1, goff:goff + gw],
                                    ps2[:1, :gw], float(c0_const), None,
                                    OP.add)

        # ------------- merged A/B chunked tanh scan pieces --------------
        # rows 1:64 = pass A chunks, rows 65:128 = pass B chunks
        def ptanh(out, z, rows):
            """tanh(z) for |z|<=0.35 as z*(1 - t/3 + 2t^2/15), t=z^2."""
            t2 = T2T[rows, :]
            g = GT[rows, :]
            nc.vector.tensor_tensor(t2, z, z, OP.mult)
            nc.vector.tensor_scalar(g, t2, 2.0 / 15.0, -1.0 / 3.0,
                                    OP.mult, OP.add)
            nc.vector.tensor_tensor(g, g, t2, OP.mult)
            nc.vector.scalar_tensor_tensor(out, g, 1.0, z, OP.add, OP.mult)

        def gather(pX, r0, r1, eng):
            # gather xp chunk rows [r0, r1) of this pass into SCT
            rlo = 1 if pX == "A" else 65
            xpr = XPR[pX]
            src = bass.AP(tensor=xpr.tensor,
                          offset=xpr.offset + CH * r0,
                          ap=[[NY, 1], [CH, r1 - r0], [1, SC]])
            eng.dma_start(SCT[rlo + r0:rlo + r1, :], src)

        def scan_prep(lo, hi):
            # mask + initial H = tanh(sct) for a 32-aligned partition range
            half = slice(lo, hi)
            nc.vector.tensor_tensor(SCT[half, :], SCT[half, :],
                                    MASK[half, :], OP.mult)
            ptanh(HT[half, 1:SC + 1], SCT[half, :], half)

        a = float(a_const)

        def scan_iter(cur, nxt):
            # Z = a*cur_shifted + sct  (leading zero col makes col0 = sct0)
            nc.vector.scalar_tensor_tensor(ZT[:, :], cur[:, 0:SC], a,
                                           SCT[:, :], OP.mult, OP.add)
            ptanh(FT[:, :], ZT[:, :], slice(0, 128))
            # G = a*(1 - F^2)
            nc.vector.tensor_tensor(GT[:, :], FT[:, :], FT[:, :], OP.mult)
            nc.vector.tensor_scalar(GT[:, :], GT[:, :], -a, a, OP.mult,
                                    OP.add)
            # d1 = F - G*cur_shifted;  nxt_t = G_t*nxt_{t-1} + d1_t
            nc.vector.tensor_tensor(T2T[:, :], GT[:, :], cur[:, 0:SC],
                                    OP.mult)
            nc.vector.tensor_tensor(DT[:, :], FT[:, :], T2T[:, :],
                                    OP.subtract)
            nc.vector.tensor_tensor_scan(nxt[:, 1:SC + 1], GT[:, :],
                                         DT[:, :], 0.0, OP.mult, OP.add)

        # ---------------- emit fc2/xp + scan schedule -------------------
        # chunk row i (0-based) reads xp cols [33i, 33i+77): i<31 needs
        # xp groups 0-1, the rest all three.  Each masked range consumes
        # at most 2 DMA-written regions (2-semaphore wait limit).
        fc2_mm("A", 0); fc2_mm("A", 1)
        xp_mm("A", 0); xp_mm("A", 1)
        gather("A", 0, 31, nc.gpsimd)
        fc2_mm("A", 2)
        xp_mm("A", 2)
        gather("A", 31, 63, nc.gpsimd)
        scan_prep(0, 64)                    # overlaps pass-B fc2 below
        fc2_mm("B", 0); fc2_mm("B", 1)
        xp_mm("B", 0); xp_mm("B", 1)
        gather("B", 0, 31, nc.sync)
        gather("B", 31, 47, nc.gpsimd)   # rows 31:47 only need groups 0-1
        fc2_mm("B", 2)
        scan_prep(64, 96)
        xp_mm("B", 2)
        gather("B", 47, 63, nc.sync)
        scan_prep(96, 128)

        cur, nxt = HT, H2T
        for it in range(SCAN_ITERS):
            scan_iter(cur, nxt)
            cur, nxt = nxt, cur

        # y = (hA - hB)/2; row r covers outputs 33*(r-1) .. +32
        hfA = cur[0:64, 1 + KW:1 + SC]
        hfB = cur[64:128, 1 + KW:1 + SC]
        nc.vector.tensor_scalar(CB[0:64, :], hfB, 0.5, None, OP.mult)
        nc.vector.scalar_tensor_tensor(D[0:64, :], hfA, 0.5, CB[0:64, :],
                                       OP.mult, OP.subtract)
        nc.sync.dma_start(
            y_d.ap()[0, 0:63 * CH].rearrange("(r c) -> r c", c=CH),
            D[1:64, :])

    nc.compile()
    return nc


def _prep_inputs(inputs):
    """Host-side packing: per-core input dicts."""
    x0 = np.asarray(inputs["x0"], np.float32)[0]
    w1 = np.asarray(inputs["conv1_w"], np.float32)
    b1 = np.asarray(inputs["conv1_b"], np.float32)
    w2 = np.asarray(inputs["conv2_w"], np.float32)
    b2 = np.asarray(inputs["conv2_b"], np.float32)
    w3 = np.asarray(inputs["conv3_w"], np.float32)
    b3 = np.asarray(inputs["conv3_b"], np.float32)
    f1w = np.asarray(inputs["fc1_w"], np.float32)
    f1b = np.asarray(inputs["fc1_b"], np.float32)
    f2w = np.asarray(inputs["fc2_w"], np.float32)
    f2b = np.asarray(inputs["fc2_b"], np.float32)
    f3w = np.asarray(inputs["fc3_w"], np.float32)
    f3b = np.asarray(inputs["fc3_b"], np.float32)
    wih = np.asarray(inputs["rnn_wih"], np.float32)
    whh = np.asarray(inputs["rnn_whh"], np.float32)
    bih = np.asarray(inputs["rnn_bih"], np.float32)
    bhh = np.asarray(inputs["rnn_bhh"], np.float32)

    a = float(whh[0, 0])
    v = (wih @ f3w)[0]
    c0 = float((wih @ f3b + bih + bhh).item())

    # W1 [11, 128]: rows 0..9 conv taps, row 10 = bias (ones-row input)
    W1 = np.zeros((11, 128), np.float32)
    for c in range(2):
        for k in range(5):
            W1[c * 5 + k, 0:64] = w1[:, c, k]
            W1[c * 5 + k, 64:128] = w1[:, c, k] * (-1.0 if c == 0 else 1.0)
    W1[10, 0:64] = b1
    W1[10, 64:128] = b1

    def pack_blockdiag(w):  # (64,64,6) -> [128, 768]
        out = np.zeros((128, 768), np.float32)
        for t in range(6):
            out[0:64, 128 * t:128 * t + 64] = w[:, :, t].T
            out[64:128, 128 * t + 64:128 * t + 128] = w[:, :, t].T
        return out

    W2B = pack_blockdiag(w2)
    W3B = pack_blockdiag(w3)

    f1r = f1w.reshape(320, 64, 7)  # flat index = ch*7 + m
    F1P = np.zeros((128, 960), np.float32)
    for p in range(3):
        F1P[0:64, 320 * p:320 * p + 320] = f1r[:, :, 2 * p].T
        F1P[64:128, 320 * p:320 * p + 320] = f1r[:, :, 2 * p + 1].T
    F1S = np.zeros((128, 320), np.float32)
    F1S[0:64, :] = f1r[:, :, 6].T
    FB1 = np.zeros((128, 3), np.float32)
    FB1[:, 0] = f1b[0:128]
    FB1[:, 1] = f1b[128:256]
    FB1[0:64, 2] = f1b[256:320]
    FB1[64:128, 2] = f1b[256:320]

    F2 = np.zeros((128, 160), np.float32)
    F2[:, 0:80] = f2w[:, 0:128].T
    F2[:, 80:160] = f2w[:, 128:256].T
    F2S2 = np.zeros((128, 80), np.float32)
    F2S2[0:64, :] = f2w[:, 256:320].T
    F2S2[64:128, :] = f2w[:, 256:320].T
    FB2 = np.zeros((128, 1), np.float32)
    FB2[0:80, 0] = f2b
    VVc = np.zeros((128, 2), np.float32)
    VVc[0:80, 0] = v

    import ml_dtypes
    bf = ml_dtypes.bfloat16
    C2 = 160 + 80 + 2
    C3 = 2 + 3 + 1 + SC + 1

    pkb = np.concatenate([F1P, F1S], axis=1).astype(bf)
    pk2 = np.concatenate([F2, F2S2, VVc], axis=1)
    assert pk2.shape == (128, C2)
    W2B = W2B.astype(bf)
    W3B = W3B.astype(bf)

    B2AB = np.concatenate([b2, b2]).reshape(128, 1)
    B3AB = np.concatenate([b3, b3]).reshape(128, 1)

    lpad = HALO
    rpad = (7 * P - HALO + NX + 8) - L
    xpad = np.zeros((2, lpad + L + max(rpad, 0)), np.float32)
    xpad[:, lpad:lpad + L] = x0

    in_maps = []
    for core in range(8):
        s = P * core
        base = lpad + s - HALO
        xww = np.zeros((11, NX + 128), np.float32)
        for c in range(2):
            for k in range(5):
                xww[c * 5 + k, 0:NX] = xpad[c, base + k:base + k + NX]
        xww[10, 0:NX] = 1.0
        xww[:, NX:NX + 128] = W1
        # scan mask: rows 0 and 64 kill garbage; rows r/64+r col j is
        # position s - 44 + 33*(r-1) + j; zero where position < 0
        mask = np.ones((128, SC), np.float32)
        mask[0, :] = 0.0
        mask[64, :] = 0.0
        if core == 0:
            for rr in range(1, 64):
                for j in range(SC):
                    if s - HALO + CH * (rr - 1) + j < 0:
                        mask[rr, j] = 0.0
                        mask[64 + rr, j] = 0.0
        pk3 = np.zeros((128, C3), np.float32)
        pk3[:, 0:1] = B2AB
        pk3[:, 1:2] = B3AB
        pk3[:, 2:5] = FB1
        pk3[:, 5:6] = FB2
        pk3[:, 6:6 + SC] = mask
        in_maps.append(dict(xww=xww, pw2=W2B, pw3=W3B, pkb=pkb, pk2=pk2,
                            pk3=pk3))
    return in_maps, a, c0


LAST_RESULT = None


def kernel(**inputs) -> np.ndarray:
    global LAST_RESULT
    from concourse import bass_utils

    in_maps, a, c0 = _prep_inputs(inputs)
    nc = _build_program(a, c0)
    res = bass_utils.run_bass_kernel_spmd(nc, in_maps, core_ids=list(range(8)))
    LAST_RESULT = res
    out = np.empty((1, W), np.float32)
    for core in range(8):
        out[0, P * core:P * core + P] = res.results[core]["y"][0][:P]
    return out
